# revision 17
# baseline (speedup 1.0000x reference)
"""DeepGCN (8-layer GCNConv, N=100k nodes, E=1.6M edges) on 8 Trainium2 cores.

Strategy (graph/data parallel, dst-sharded edges):
  - Nodes are degree-sorted and dealt serpentine-wise across the 8 cores so
    every core owns n/8 nodes with a near-identical degree profile.  Within a
    core, nodes are packed into "supergroups" of 1024 = 128 partitions x 8
    degree-band groups (group g = the g-th 128 nodes by degree rank), and the
    per-edge slot table is rectangular per (supergroup, group) with height =
    that band's max in-degree (uniform across cores), within 1.5% of the
    true edge count.
  - Each layer writes a bf16 feature table row per node, pre-scaled by
    dinv[node] (the src half of the GCN norm), with one all-zero pad row per
    shard.  An AllGather assembles the full table on every core.
  - Message passing: the table is viewed as 256-byte "quad rows" of 4
    consecutive nodes, and edges are fetched with dma_gather (int16 quad-row
    indices, up to 1024 per call from 128-byte-aligned index windows, spread
    over 4 SWDGE queues congruently with the Tile scheduler's 8-lane DMA
    semaphore rotation so the Q7 descriptor generation pipelines).  A per-slot {0,1} mask (broadcast along the
    feature axis) selects the right node out of each gathered quad, then a
    strided DVE tensor_reduce folds the 8 slots x 4 subblocks into the
    per-destination partial sum.  dinv[dst] is applied after the reduce.
  - GCNConv is computed aggregate-first:  A(xW) == (Ax)W.  The 32x32 weight
    is applied as a block-diagonal 4x(32x32) 128x128 matmul on PE after a
    128x128 PE transpose; bias+relu fuse into the PSUM->SBUF copy on ACT.
  - Residual + dropout-mask multiply + next-layer table write all happen in
    node-major [128, 8*32] tiles on DVE.
"""

import numpy as np
import ml_dtypes

import concourse.bass as bass
import concourse.bacc as bacc
import concourse.mybir as mybir
import concourse.tile as tile
from concourse.bass import broadcast_tensor_aps
from concourse.bass_utils import run_bass_kernel_spmd
from concourse.masks import make_identity

N_CORES = 8
P = 128
G = 8            # groups (nodes) per partition-row of one supergroup
SGN = P * G      # nodes per supergroup
CW = 8           # gather-call width: 8 slot-columns = 1024 indices
F32 = mybir.dt.float32
BF16 = mybir.dt.bfloat16
I32 = mybir.dt.int32
I16 = mybir.dt.int16
AX = mybir.AxisListType
ALU = mybir.AluOpType
ACTF = mybir.ActivationFunctionType


# ---------------------------------------------------------------- host prep
def _build_structure(edge_index, n):
    """Degree-sorted serpentine node partition + per-core slot tables.

    Slot table: per supergroup sg, per degree-band group g, kgg[sg,g] slot
    columns (col = goff[sg*G+g] + j).  Each slot holds the int16 quad-row id
    (new_id >> 2) of its source node; a per-slot one-hot bf16 mask over the
    4 subblocks encodes new_id & 3.  Slot columns are consumed by dma_gather
    in calls of <= CW=8 columns (<=1024 indices, wrapped over 16 partitions
    in 128B-aligned windows).
    """
    E = edge_index.shape[1]
    dst_all = np.concatenate([edge_index[1].astype(np.int64), np.arange(n)])
    deg = np.bincount(dst_all, minlength=n)
    dinv = (1.0 / np.sqrt(deg)).astype(np.float32)

    order = np.argsort(-deg, kind="stable")
    idx = np.arange(n)
    rounds, pos = idx // N_CORES, idx % N_CORES
    cores_seq = np.where(rounds % 2 == 0, pos, N_CORES - 1 - pos)
    core_of = np.empty(n, dtype=np.int32)
    core_of[order] = cores_seq
    local_rank = np.empty(n, dtype=np.int64)
    for c in range(N_CORES):
        nodes_c = order[cores_seq == c]
        local_rank[nodes_c] = np.arange(len(nodes_c))

    n_local = n // N_CORES
    n_sg = (n_local + SGN - 1) // SGN
    npad = n_sg * SGN
    new_id = core_of.astype(np.int64) * npad + local_rank
    pad_q = n_local >> 2                  # core 0's zero pad region, quad row

    edge_dst = edge_index[1].astype(np.int64)
    edge_src = edge_index[0].astype(np.int64)
    per_core = []
    kmat = np.zeros((N_CORES, n_sg), dtype=np.int64)
    for c in range(N_CORES):
        em = core_of[edge_dst] == c
        e_src = new_id[edge_src[em]]
        e_rank = local_rank[edge_dst[em]]
        o = np.argsort(e_rank, kind="stable")
        e_src, e_rank = e_src[o], e_rank[o]
        counts = np.bincount(e_rank, minlength=n_local)
        starts = np.concatenate([[0], np.cumsum(counts)])
        per_core.append((e_src, e_rank, counts, starts))
        for sg in range(n_sg):
            kmat[c, sg] = counts[sg * SGN : min((sg + 1) * SGN, n_local)].max()
    # per-(sg, g) rectangle heights, max over the 128 dsts and all cores
    kgg = np.zeros((N_CORES, n_sg, G), dtype=np.int64)
    for c in range(N_CORES):
        _, e_rank, _, _ = per_core[c]
        sg = e_rank // SGN
        i = e_rank % SGN
        key = (sg * G + (i // P)) * P + (i % P)
        cnt = np.bincount(key, minlength=n_sg * G * P).reshape(n_sg * G, P)
        kgg[c] = cnt.max(axis=1).reshape(n_sg, G)
    kgg = kgg.max(axis=0)                        # [n_sg, G], SPMD-uniform
    goff = np.concatenate([[0], np.cumsum(kgg.ravel())]).reshape(-1)
    cols_total = int(goff[-1])

    # call list: per (sg, g), chunks of <= CW columns; width w -> 128*w idxs
    calls = []                                   # (sg, g, colbase, width)
    for sg in range(n_sg):
        for g in range(G):
            k = int(kgg[sg, g])
            base = int(goff[sg * G + g])
            off = 0
            while off < k:
                w = min(CW, k - off)
                calls.append((sg, g, base + off, w))
                off += w
    iw_off = np.arange(len(calls) + 1) * 64      # 128B-aligned idx windows
    idx_words = int(iw_off[-1])                  # int16 words per partition

    slots4 = np.full((N_CORES, P, cols_total), pad_q, dtype=np.int16)
    msel = np.zeros((N_CORES, P, 4 * cols_total), dtype=ml_dtypes.bfloat16)
    idx4 = np.zeros((N_CORES, P, idx_words + 8), dtype=np.int16)
    for c in range(N_CORES):
        e_src, e_rank, counts, starts = per_core[c]
        j = np.arange(len(e_src)) - starts[e_rank]      # slot within node
        sg = e_rank // SGN
        i = e_rank % SGN
        pp, gg = i % P, i // P
        col = goff[sg * G + gg] + j
        slots4[c, pp, col] = (e_src >> 2).astype(np.int16)
        msel[c, pp, 4 * col + (e_src & 3)] = 1.0
        # wrapped indices per call: position j = colrel*128 + p -> [j%16, j//16]
        blk = np.zeros((16, idx_words), dtype=np.int16)
        for t, (csg, cg, cbase, w) in enumerate(calls):
            arr = slots4[c, :, cbase : cbase + w].T.ravel()   # [128*w]
            blk[:, int(iw_off[t]) : int(iw_off[t]) + 8 * w] = (
                arr.reshape(8 * w, 16).T)
        idx4[c, :, :idx_words] = np.tile(blk, (8, 1))
        # trailing all-zero window: observer gathers read row 0 safely
    return (core_of, local_rank, dinv, idx4, msel, calls, iw_off, cols_total,
            n_local, n_sg)


# ------------------------------------------------------------- bass program
def _build_program(n_local, n_sg, calls, iw_off, cols_total, f_in, n_cls,
                   n_hidden_layers):
    """n_hidden_layers = number of 32->32 convs (6 for the real problem)."""
    F = 32
    npad = n_sg * SGN
    ntab = N_CORES * npad
    idx_words = int(iw_off[-1])
    n_layers = n_hidden_layers + 2        # conv0 + hidden + output conv
    NQ = 4                                # SWDGE queues for dma_gather

    nc = bacc.Bacc(num_devices=N_CORES, num_swdge_queues=NQ,
                   dynamic_dma_scratch_size=65536)
    xT_p = nc.declare_dram_parameter("xT", [f_in, npad], F32, False)
    idx4_p = nc.declare_dram_parameter("idx4", [P, idx_words + 8], I16, False)
    msel_p = nc.declare_dram_parameter("msel", [P, 4 * cols_total], BF16,
                                       False)
    dinvb_p = nc.declare_dram_parameter("dinvb", [P, n_sg * G * F], F32, False)
    mask_p = nc.declare_dram_parameter(
        "mask", [n_layers - 1, n_sg, P, G * F], BF16, False)
    w0_p = nc.declare_dram_parameter("W0", [f_in, F], F32, False)
    if n_hidden_layers:
        w4_p = nc.declare_dram_parameter("W4", [n_hidden_layers, P, P], F32, False)
        b4_p = nc.declare_dram_parameter("b4", [P, n_hidden_layers], F32, False)
    b0t_p = nc.declare_dram_parameter("b0t", [P, G * F], F32, False)
    wout_p = nc.declare_dram_parameter("Wout", [P, n_cls], F32, False)
    bout_p = nc.declare_dram_parameter("bout", [n_cls, 1], F32, False)
    out_p = nc.declare_dram_parameter("outT", [n_cls, npad], F32, True)

    rg = [list(range(N_CORES))]

    with tile.TileContext(nc) as tc:
        import contextlib
        with contextlib.ExitStack() as ctx:
            const = ctx.enter_context(tc.tile_pool(name="const", bufs=1))
            dram = ctx.enter_context(
                tc.tile_pool(name="dramp", bufs=1, space="DRAM"))
            psum = ctx.enter_context(
                tc.tile_pool(name="psum", bufs=4, space="PSUM"))
            pso = ctx.enter_context(
                tc.tile_pool(name="pso", bufs=2, space="PSUM"))
            pscrap = ctx.enter_context(
                tc.tile_pool(name="pscrap", bufs=1, space="PSUM"))
            sb = ctx.enter_context(tc.tile_pool(name="sb", bufs=3))
            sb2 = ctx.enter_context(tc.tile_pool(name="sb2", bufs=4))
            gat = ctx.enter_context(tc.tile_pool(name="gat", bufs=10))
            sgp = ctx.enter_context(tc.tile_pool(name="sgp", bufs=2))
            xts = ctx.enter_context(tc.tile_pool(name="xts", bufs=2))

            # persistent tiles
            idx4_sb = const.tile([P, idx_words + 8], I16, name="idx4_sb")
            nc.sync.dma_start(out=idx4_sb[:], in_=idx4_p[:])
            msel_sb = const.tile([P, 4 * cols_total], BF16, name="msel_sb")
            nc.sync.dma_start(out=msel_sb[:], in_=msel_p[:])
            dinvb = const.tile([P, n_sg * G * F], F32, name="dinvb_sb")
            nc.sync.dma_start(out=dinvb[:], in_=dinvb_p[:])
            w0_sb = const.tile([f_in, F], F32, name="w0_sb")
            nc.sync.dma_start(out=w0_sb[:], in_=w0_p[:])
            if n_hidden_layers:
                w4_sb = const.tile([P, n_hidden_layers, P], F32, name="w4_sb")
                nc.sync.dma_start(
                    out=w4_sb[:], in_=w4_p[:].rearrange("l k m -> k l m"))
                b4_sb = const.tile([P, n_hidden_layers], F32, name="b4_sb")
                nc.sync.dma_start(out=b4_sb[:], in_=b4_p[:])
            b0t_sb = const.tile([P, G * F], F32, name="b0t_sb")
            nc.sync.dma_start(out=b0t_sb[:], in_=b0t_p[:])
            wout_sb = const.tile([P, n_cls], F32, name="wout_sb")
            nc.sync.dma_start(out=wout_sb[:], in_=wout_p[:])
            bout_sb = const.tile([n_cls, 1], F32, name="bout_sb")
            nc.sync.dma_start(out=bout_sb[:], in_=bout_p[:])
            ident = const.tile([P, P], F32, name="ident_sb")
            make_identity(nc, ident[:])
            xold = const.tile([P, n_sg * G * F], F32, name="xold_sb")
            nc.vector.memset(xold[:], 0.0)
            stag = const.tile([P, n_sg * G * F], BF16, name="stag_sb")
            scrap_ps = pscrap.tile([32, 32], F32, name="scrapps_sb")
            scrap_dve = const.tile([1, 8], F32, name="scrapdve_sb")
            scrap_dve2 = const.tile([1, 8], F32, name="scrapdve2_sb")
            scrap_dve3 = const.tile([1, 8], F32, name="scrapdve3_sb")
            scrap_dve4 = const.tile([1, 8], BF16, name="scrapdve4_sb")
            scrap_act = const.tile([1, 8], F32, name="scrapact_sb")

            pool_ord = [0]

            def gq():
                q = pool_ord[0] % NQ
                pool_ord[0] += 1
                return q

            obs_idx = None     # set after idx4_sb load: zero idx window
            shard = dram.tile([npad, F], BF16, name="shard_d")
            tables = [
                dram.tile([ntab, F], BF16, name=f"tab{i}_d", addr_space="Shared")
                for i in range(n_layers)]

            # --- startup observers: absorb const-load DMA ticks per engine
            nc.tensor.transpose(out=scrap_ps[:], in_=ident[0:32, 0:32],
                                identity=ident[0:32, 0:32])
            nc.tensor.transpose(out=scrap_ps[:], in_=w0_sb[0:32, 0:32],
                                identity=ident[0:32, 0:32])
            if n_hidden_layers:
                nc.tensor.transpose(out=scrap_ps[:], in_=w4_sb[0:32, 0, 0:32],
                                    identity=ident[0:32, 0:32])
                nc.scalar.activation(out=scrap_act[:, 0:1], in_=b4_sb[0:1, 0:1],
                                     func=ACTF.Copy)
            nc.tensor.transpose(out=scrap_ps[:], in_=wout_sb[0:32, 0:32],
                                identity=ident[0:32, 0:32])
            nc.vector.tensor_copy(out=scrap_dve[:, 0:1], in_=dinvb[0:1, 0:1])
            nc.vector.tensor_copy(out=scrap_dve2[:, 0:1], in_=b0t_sb[0:1, 0:1])
            nc.vector.tensor_copy(out=scrap_dve3[:, 0:1], in_=bout_sb[0:1, 0:1])
            nc.vector.tensor_copy(out=scrap_dve4[:, 0:1], in_=msel_sb[0:1, 0:1])
            obs_idx = idx4_sb[:, idx_words : idx_words + 8]
            maskview = mask_p[:].rearrange("a b p (f2 e) -> (a b p f2) e", e=128)
            sg0 = sgp.tile([P, 128], BF16, tag="obs", name="obs_start")
            nc.gpsimd.dma_gather(
                sg0[:].rearrange("p (c e) -> p c e", e=128), maskview,
                obs_idx, P, P, 128, queue_num=gq())

            def observe_table(li, tab):
                # absorb the collective-done tick on SP and Pool
                ssp = const.tile([1, F], BF16, name=f"obs_sp{li}")
                nc.sync.dma_start(out=ssp[:], in_=tab[0:1, :])
                tab4v = tab[:].rearrange("(r q) f -> r (q f)", q=4)
                so = sgp.tile([P, 128], BF16, tag="obs", name=f"obs_pl{li}")
                nc.gpsimd.dma_gather(
                    so[:].rearrange("p (c e) -> p c e", e=128), tab4v,
                    obs_idx, P, P, 128, queue_num=gq())

            def flush_sg(sg):
                """Stage one supergroup's slice of the shard early."""
                nc.sync.dma_start(
                    out=shard[sg * SGN : (sg + 1) * SGN, :].rearrange(
                        "(g p) f -> p g f", g=G, p=P),
                    in_=stag[:, sg * G * F : (sg + 1) * G * F].rearrange(
                        "p (g f) -> p g f", f=F))

            def flush_and_gather(li):
                """AllGather the (already staged) shard into tables[li]."""
                nc.gpsimd.collective_compute(
                    "AllGather", ALU.bypass, replica_groups=rg,
                    ins=[shard.opt()], outs=[tables[li].opt()])
                observe_table(li, tables[li])

            def psg_of(sg):
                return P

            # ---------------- conv 0: h0 = x @ W0, staging <- dinv * h0
            for sg in range(n_sg):
                xt = xts.tile([f_in, SGN], F32, tag="xt", name=f"xt{sg}")
                nc.sync.dma_start(
                    out=xt[:], in_=xT_p[:, sg * SGN : (sg + 1) * SGN])
                # absorb the xt DMA tick on PE before the real matmuls
                nc.tensor.transpose(out=scrap_ps[:], in_=xt[0:32, 0:32],
                                    identity=ident[0:32, 0:32])
                ps = psum.tile([P, G * F], F32, tag="ps", name=f"c0ps{sg}")
                for g in range(G):
                    nc.tensor.matmul(
                        out=ps[:, g * F : (g + 1) * F],
                        lhsT=xt[:, g * P : (g + 1) * P],
                        rhs=w0_sb[:], start=True, stop=True)
                nc.vector.tensor_tensor(
                    out=stag[:, sg * G * F : (sg + 1) * G * F], in0=ps[:],
                    in1=dinvb[:, sg * G * F : (sg + 1) * G * F], op=ALU.mult)
                flush_sg(sg)
            flush_and_gather(0)

            # ---------------- convs 1..n_layers
            sg_call_ranges = []
            for sg in range(n_sg):
                ts = [t for t, cc in enumerate(calls) if cc[0] == sg]
                sg_call_ranges.append((min(ts), max(ts) + 1) if ts else (0, 0))
            for li in range(1, n_layers + 1):
                tab = tables[li - 1]
                tab4 = tab[:].rearrange("(r q) f -> r (q f)", q=4)
                hidden = li < n_layers
                for sg in range(n_sg):
                    psg = psg_of(sg)
                    agg = sb.tile([P, G * F], F32, tag="agg", name=f"ag{li}_{sg}")
                    t0, t1 = sg_call_ranges[sg]
                    bands = {calls[t][1] for t in range(t0, t1)}
                    for g in range(G):
                        # empty degree bands (tail supergroup): keep finite
                        if g not in bands:
                            nc.vector.memset(agg[:, g * F : (g + 1) * F], 0.0)
                    seen = set()
                    for tcall in range(t0, t1):
                        _, g, cbase, w = calls[tcall]
                        iw = int(iw_off[tcall])
                        gt = gat.tile([P, CW * 128], BF16, tag="gt",
                                      name=f"gt{li}_{tcall}")
                        nc.gpsimd.dma_gather(
                            gt[:, : w * 128].rearrange(
                                "p (c e) -> p c e", e=128),
                            tab4,
                            idx4_sb[:, iw : iw + 8 * w],
                            w * P, w * P, 128,
                            queue_num=gq())
                        prod = sb2.tile([P, CW * 128], BF16, tag="prod",
                                        name=f"pr{li}_{tcall}")
                        in0 = gt[:, : w * 128].rearrange(
                            "p (cb f) -> p cb f", f=F)
                        in1 = msel_sb[:, 4 * cbase : 4 * (cbase + w)].rearrange(
                            "p cb -> p cb ()")
                        i0b, i1b = broadcast_tensor_aps(in0, in1)
                        nc.vector.tensor_tensor(
                            out=prod[:, : w * 128].rearrange(
                                "p (cb f) -> p cb f", f=F),
                            in0=i0b, in1=i1b, op=ALU.mult)
                        red_in = prod[:, : w * 128].rearrange(
                            "p (cb f) -> p f cb", f=F)
                        if g not in seen:
                            # first call of this (sg, g) rect writes agg direct
                            seen.add(g)
                            nc.vector.tensor_reduce(
                                out=agg[:, g * F : (g + 1) * F],
                                in_=red_in, axis=AX.X, op=ALU.add)
                        else:
                            part = sb2.tile([P, F], F32, tag="part",
                                            name=f"pt{li}_{tcall}")
                            nc.vector.tensor_reduce(
                                out=part[:], in_=red_in, axis=AX.X, op=ALU.add)
                            nc.vector.tensor_tensor(
                                out=agg[:, g * F : (g + 1) * F],
                                in0=agg[:, g * F : (g + 1) * F],
                                in1=part[:], op=ALU.add)
                    dv = dinvb[:, sg * G * F : (sg + 1) * G * F]
                    r_nm = sb.tile([P, G * F], F32, tag="rnm", name=f"rn{li}_{sg}")
                    # self-loop term (staging still holds last layer's dinv*x)
                    nc.vector.tensor_tensor(
                        out=r_nm[:], in0=agg[:],
                        in1=stag[:, sg * G * F : (sg + 1) * G * F], op=ALU.add)
                    nc.vector.tensor_tensor(
                        out=r_nm[:psg], in0=r_nm[:psg], in1=dv[:psg], op=ALU.mult)
                    if li == 1:
                        # conv0 epilogue: t = relu(agg*dinv + b0); xold=t
                        nc.vector.tensor_tensor(
                            out=r_nm[:psg], in0=r_nm[:psg], in1=b0t_sb[:psg],
                            op=ALU.add)
                        xsl = xold[:psg, sg * G * F : (sg + 1) * G * F]
                        nc.vector.tensor_scalar_max(xsl, r_nm[:psg], 0.0)
                        _emit_mask_and_write(
                            nc, sb, mask_p, 0, sg, xold, dv, stag, F)
                        flush_sg(sg)
                        continue
                    # transpose r -> rT (feat-major strips)
                    nh = (G * F) // P             # transpose halves (=2)
                    pst = psum.tile([P, G * F], F32, tag="ps", name=f"pt{li}_{sg}")
                    for h in range(nh):
                        nc.tensor.transpose(
                            out=pst[:, h * P : h * P + psg],
                            in_=r_nm[:psg, h * P : (h + 1) * P],
                            identity=ident[:psg, :psg])
                    rt = sb.tile([P, G * F], F32, tag="rt", name=f"rt{li}_{sg}")
                    for h in range(nh):
                        nc.vector.tensor_copy(
                            out=rt[:, h * P : h * P + psg],
                            in_=pst[:, h * P : h * P + psg])
                    if hidden:
                        hw = li - 2
                        psh = psum.tile([P, G * F], F32, tag="ps",
                                        name=f"ph{li}_{sg}")
                        for h in range(nh):
                            nc.tensor.matmul(
                                out=psh[:, h * P : h * P + psg],
                                lhsT=w4_sb[:, hw, :],
                                rhs=rt[:, h * P : h * P + psg],
                                start=True, stop=True)
                        ht = sb.tile([P, G * F], F32, tag="ht",
                                     name=f"ht{li}_{sg}")
                        for h in range(nh):
                            nc.scalar.activation(
                                out=ht[:, h * P : h * P + psg],
                                in_=psh[:, h * P : h * P + psg],
                                func=ACTF.Relu, bias=b4_sb[:, hw : hw + 1])
                        psb = psum.tile([P, G * F], F32, tag="ps",
                                        name=f"pb{li}_{sg}")
                        for h in range(nh):
                            nc.tensor.transpose(
                                out=psb[:psg, h * P : (h + 1) * P],
                                in_=ht[:, h * P : h * P + psg],
                                identity=ident[:])
                        xsl = xold[:psg, sg * G * F : (sg + 1) * G * F]
                        nc.vector.tensor_tensor(
                            out=xsl, in0=psb[:psg], in1=xsl, op=ALU.add)
                        _emit_mask_and_write(
                            nc, sb, mask_p, li - 1, sg, xold, dv, stag, F)
                        flush_sg(sg)
                    else:
                        # output conv: out strips = Wout^T @ rT quads
                        for g in range(G):
                            h, i = g // 4, g % 4
                            if h >= nh:
                                continue
                            po = pso.tile([n_cls, P], F32, tag="po",
                                          name=f"po{sg}_{g}")
                            nc.tensor.matmul(
                                out=po[:, :psg],
                                lhsT=wout_sb[i * F : (i + 1) * F, :],
                                rhs=rt[i * F : (i + 1) * F, h * P : h * P + psg],
                                start=True, stop=True,
                                tile_position=(i * F, 0))
                            ot = sb.tile([n_cls, P], F32, tag="ot",
                                         name=f"ot{sg}_{g}")
                            nc.vector.tensor_scalar_add(
                                ot[:, :psg], po[:, :psg], bout_sb[:, 0:1])
                            nc.sync.dma_start(
                                out=out_p[:, sg * SGN + g * P : sg * SGN + g * P + psg],
                                in_=ot[:, :psg])
                if hidden:
                    flush_and_gather(li)
    if not nc.is_finalized():
        nc.finalize()
    return nc


def _emit_mask_and_write(nc, sb, mask_p, li, sg, xold, dv, stag, F):
    """xd = xold_slice * mask[li,sg]; staging <- bf16(xd * dinv).

    Full 128-partition ops: pad rows give 0 (xold memset, mask/dinv
    host-zeroed), which makes the shard pad rows exactly zero.
    """
    mt = sb.tile([P, G * F], BF16, tag="mt", name=f"mt{li}_{sg}")
    nc.sync.dma_start(out=mt[:], in_=mask_p[li, sg, :, :])
    xsl = xold[:, sg * G * F : (sg + 1) * G * F]
    xd = sb.tile([P, G * F], F32, tag="xd", name=f"xd{li}_{sg}")
    nc.vector.tensor_tensor(out=xd[:], in0=xsl, in1=mt[:], op=ALU.mult)
    nc.vector.tensor_tensor(out=stag[:, sg * G * F : (sg + 1) * G * F],
                            in0=xd[:], in1=dv, op=ALU.mult)


# ------------------------------------------------------------------ driver
def _host_inputs(x, edge_index, drop_u, W0, b0, W_hid, b_hid, W_out, b_out,
                 struct):
    (core_of, local_rank, dinv, idx4, msel, calls, iw_off, cols_total,
     n_local, n_sg) = struct
    n, f_in = x.shape
    F = 32
    n_cls = W_out.shape[1]
    nhid = W_hid.shape[0]
    npad = n_sg * SGN
    n_masks = drop_u.shape[0]

    # rank -> old id per core; xT col order (sg, g, p): col -> rank
    cols = np.arange(npad)
    csg, cj = cols // SGN, cols % SGN
    cg, cp = cj // P, cj % P
    rank_of_col = csg * SGN + cg * P + cp          # may exceed n_local (pad)
    col_valid = rank_of_col < n_local

    w4 = np.zeros((nhid, P, P), dtype=np.float32)
    for i in range(nhid):
        for q in range(4):
            w4[i, q * F : (q + 1) * F, q * F : (q + 1) * F] = W_hid[i]
    b4 = np.tile(b_hid.T, (4, 1)).astype(np.float32) if nhid else None  # [128, nhid]

    in_maps = []
    for c in range(N_CORES):
        nodes_c = np.where(core_of == c)[0]
        r = local_rank[nodes_c]
        ordmap = np.empty(n_local, dtype=np.int64)
        ordmap[r] = nodes_c

        xT = np.zeros((f_in, npad), dtype=np.float32)
        xT[:, col_valid] = x[ordmap[rank_of_col[col_valid]]].T

        dvals = np.zeros(npad, dtype=np.float32)
        dvals[:n_local] = dinv[ordmap]
        dinvb = np.repeat(
            dvals.reshape(n_sg, G, P).transpose(2, 0, 1), F,
            axis=2).reshape(P, n_sg * G * F).copy()

        mask = np.zeros((n_masks, n_sg, P, G * F), dtype=ml_dtypes.bfloat16)
        mvals = np.where(drop_u[:, ordmap, :] > 0.5, 2.0, 0.0).astype(
            ml_dtypes.bfloat16)                      # [n_masks, n_local, F]
        mpad = np.zeros((n_masks, npad, F), dtype=ml_dtypes.bfloat16)
        mpad[:, :n_local] = mvals
        mask[:] = mpad.reshape(n_masks, n_sg, G, P, F).transpose(
            0, 1, 3, 2, 4).reshape(n_masks, n_sg, P, G * F)

        im = {
            "xT": xT,
            "idx4": idx4[c],
            "msel": msel[c],
            "dinvb": dinvb,
            "mask": mask,
            "W0": W0.astype(np.float32),
            "b0t": np.tile(b0, (P, G)).astype(np.float32),
            "Wout": np.tile(W_out, (4, 1)).astype(np.float32),
            "bout": b_out.reshape(-1, 1).astype(np.float32),
        }
        if nhid:
            im["W4"] = w4
            im["b4"] = b4
        in_maps.append(im)
    return in_maps


def kernel(x, edge_index, drop_u, W0, b0, W_hid, b_hid, W_out, b_out,
           _runner=None):
    x = np.asarray(x, dtype=np.float32)
    edge_index = np.asarray(edge_index)
    drop_u = np.asarray(drop_u, dtype=np.float32)
    W0 = np.asarray(W0, dtype=np.float32)
    b0 = np.asarray(b0, dtype=np.float32)
    W_hid = np.asarray(W_hid, dtype=np.float32)
    b_hid = np.asarray(b_hid, dtype=np.float32)
    W_out = np.asarray(W_out, dtype=np.float32)
    b_out = np.asarray(b_out, dtype=np.float32)

    n, f_in = x.shape
    n_cls = W_out.shape[1]
    struct = _build_structure(edge_index, n)
    (core_of, local_rank, dinv, idx4, msel, calls, iw_off, cols_total,
     n_local, n_sg) = struct

    nc = _build_program(n_local, n_sg, calls, iw_off, cols_total, f_in,
                        n_cls, W_hid.shape[0])
    in_maps = _host_inputs(x, edge_index, drop_u, W0, b0, W_hid, b_hid,
                           W_out, b_out, struct)

    if _runner is not None:
        results = _runner(nc, in_maps)
    else:
        results = run_bass_kernel_spmd(
            nc, in_maps, core_ids=list(range(N_CORES))).results

    # un-permute: outT [n_cls, npad] per core, col -> rank -> old id
    npad = n_sg * SGN
    cols = np.arange(npad)
    csg, cj = cols // SGN, cols % SGN
    cg, cp = cj // P, cj % P
    rank_of_col = csg * SGN + cg * P + cp
    col_valid = rank_of_col < n_local

    out = np.zeros((n, n_cls), dtype=np.float32)
    for c in range(N_CORES):
        nodes_c = np.where(core_of == c)[0]
        r = local_rank[nodes_c]
        ordmap = np.empty(n_local, dtype=np.int64)
        ordmap[r] = nodes_c
        ot = np.asarray(results[c]["outT"], dtype=np.float32)  # [n_cls, npad]
        out[ordmap[rank_of_col[col_valid]]] = ot[:, col_valid].T
    return out


# revision 18
# speedup vs baseline: 1.0126x; 1.0126x over previous
"""DeepGCN (8-layer GCNConv, N=100k nodes, E=1.6M edges) on 8 Trainium2 cores.

Strategy (graph/data parallel, dst-sharded edges):
  - Nodes are degree-sorted and dealt serpentine-wise across the 8 cores so
    every core owns n/8 nodes with a near-identical degree profile.  Within a
    core, nodes are packed into "supergroups" of 1024 = 128 partitions x 8
    degree-band groups (group g = the g-th 128 nodes by degree rank), and the
    per-edge slot table is rectangular per (supergroup, group) with height =
    that band's max in-degree (uniform across cores), within 1.5% of the
    true edge count.
  - Each layer writes a bf16 feature table row per node, pre-scaled by
    dinv[node] (the src half of the GCN norm), with one all-zero pad row per
    shard.  An AllGather assembles the full table on every core.
  - Message passing: the table is viewed as 256-byte "quad rows" of 4
    consecutive nodes, and edges are fetched with dma_gather (int16 quad-row
    indices, up to 1024 per call from 128-byte-aligned index windows, spread
    over 4 SWDGE queues congruently with the Tile scheduler's 8-lane DMA
    semaphore rotation so the Q7 descriptor generation pipelines).  A per-slot {0,1} mask (broadcast along the
    feature axis) selects the right node out of each gathered quad, then a
    strided DVE tensor_reduce folds the 8 slots x 4 subblocks into the
    per-destination partial sum.  dinv[dst] is applied after the reduce.
  - GCNConv is computed aggregate-first:  A(xW) == (Ax)W.  The 32x32 weight
    is applied as a block-diagonal 4x(32x32) 128x128 matmul on PE after a
    128x128 PE transpose; bias+relu fuse into the PSUM->SBUF copy on ACT.
  - Residual + dropout-mask multiply + next-layer table write all happen in
    node-major [128, 8*32] tiles on DVE.
"""

import numpy as np
import ml_dtypes

import concourse.bass as bass
import concourse.bacc as bacc
import concourse.mybir as mybir
import concourse.tile as tile
from concourse.bass import broadcast_tensor_aps
from concourse.bass_utils import run_bass_kernel_spmd
from concourse.masks import make_identity

N_CORES = 8
P = 128
G = 8            # groups (nodes) per partition-row of one supergroup
SGN = P * G      # nodes per supergroup
CW = 8           # gather-call width: 8 slot-columns = 1024 indices
F32 = mybir.dt.float32
BF16 = mybir.dt.bfloat16
I32 = mybir.dt.int32
I16 = mybir.dt.int16
AX = mybir.AxisListType
ALU = mybir.AluOpType
ACTF = mybir.ActivationFunctionType


# ---------------------------------------------------------------- host prep
def _build_structure(edge_index, n):
    """Degree-sorted serpentine node partition + per-core slot tables.

    Slot table: per supergroup sg, per degree-band group g, kgg[sg,g] slot
    columns (col = goff[sg*G+g] + j).  Each slot holds the int16 quad-row id
    (new_id >> 2) of its source node; a per-slot one-hot bf16 mask over the
    4 subblocks encodes new_id & 3.  Slot columns are consumed by dma_gather
    in calls of <= CW=8 columns (<=1024 indices, wrapped over 16 partitions
    in 128B-aligned windows).
    """
    E = edge_index.shape[1]
    dst_all = np.concatenate([edge_index[1].astype(np.int64), np.arange(n)])
    deg = np.bincount(dst_all, minlength=n)
    dinv = (1.0 / np.sqrt(deg)).astype(np.float32)

    order = np.argsort(-deg, kind="stable")
    idx = np.arange(n)
    rounds, pos = idx // N_CORES, idx % N_CORES
    cores_seq = np.where(rounds % 2 == 0, pos, N_CORES - 1 - pos)
    core_of = np.empty(n, dtype=np.int32)
    core_of[order] = cores_seq
    local_rank = np.empty(n, dtype=np.int64)
    for c in range(N_CORES):
        nodes_c = order[cores_seq == c]
        local_rank[nodes_c] = np.arange(len(nodes_c))

    n_local = n // N_CORES
    n_sg = (n_local + SGN - 1) // SGN
    npad = n_sg * SGN
    new_id = core_of.astype(np.int64) * npad + local_rank
    pad_q = n_local >> 2                  # core 0's zero pad region, quad row

    edge_dst = edge_index[1].astype(np.int64)
    edge_src = edge_index[0].astype(np.int64)
    per_core = []
    kmat = np.zeros((N_CORES, n_sg), dtype=np.int64)
    for c in range(N_CORES):
        em = core_of[edge_dst] == c
        e_src = new_id[edge_src[em]]
        e_rank = local_rank[edge_dst[em]]
        o = np.argsort(e_rank, kind="stable")
        e_src, e_rank = e_src[o], e_rank[o]
        counts = np.bincount(e_rank, minlength=n_local)
        starts = np.concatenate([[0], np.cumsum(counts)])
        per_core.append((e_src, e_rank, counts, starts))
        for sg in range(n_sg):
            kmat[c, sg] = counts[sg * SGN : min((sg + 1) * SGN, n_local)].max()
    # per-(sg, g) rectangle heights, max over the 128 dsts and all cores
    kgg = np.zeros((N_CORES, n_sg, G), dtype=np.int64)
    for c in range(N_CORES):
        _, e_rank, _, _ = per_core[c]
        sg = e_rank // SGN
        i = e_rank % SGN
        key = (sg * G + (i // P)) * P + (i % P)
        cnt = np.bincount(key, minlength=n_sg * G * P).reshape(n_sg * G, P)
        kgg[c] = cnt.max(axis=1).reshape(n_sg, G)
    kgg = kgg.max(axis=0)                        # [n_sg, G], SPMD-uniform
    goff = np.concatenate([[0], np.cumsum(kgg.ravel())]).reshape(-1)
    cols_total = int(goff[-1])

    # call list: per (sg, g), chunks of <= CW columns; width w -> 128*w idxs
    calls = []                                   # (sg, g, colbase, width)
    for sg in range(n_sg):
        for g in range(G):
            k = int(kgg[sg, g])
            base = int(goff[sg * G + g])
            off = 0
            while off < k:
                w = min(CW, k - off)
                calls.append((sg, g, base + off, w))
                off += w
    iw_off = np.arange(len(calls) + 1) * 64      # 128B-aligned idx windows
    idx_words = int(iw_off[-1])                  # int16 words per partition

    slots4 = np.full((N_CORES, P, cols_total), pad_q, dtype=np.int16)
    msel = np.zeros((N_CORES, P, 4 * cols_total), dtype=ml_dtypes.bfloat16)
    idx4 = np.zeros((N_CORES, P, idx_words + 8), dtype=np.int16)
    for c in range(N_CORES):
        e_src, e_rank, counts, starts = per_core[c]
        j = np.arange(len(e_src)) - starts[e_rank]      # slot within node
        sg = e_rank // SGN
        i = e_rank % SGN
        pp, gg = i % P, i // P
        col = goff[sg * G + gg] + j
        slots4[c, pp, col] = (e_src >> 2).astype(np.int16)
        msel[c, pp, 4 * col + (e_src & 3)] = 1.0
        # wrapped indices per call: position j = colrel*128 + p -> [j%16, j//16]
        blk = np.zeros((16, idx_words), dtype=np.int16)
        for t, (csg, cg, cbase, w) in enumerate(calls):
            arr = slots4[c, :, cbase : cbase + w].T.ravel()   # [128*w]
            blk[:, int(iw_off[t]) : int(iw_off[t]) + 8 * w] = (
                arr.reshape(8 * w, 16).T)
        idx4[c, :, :idx_words] = np.tile(blk, (8, 1))
        # trailing all-zero window: observer gathers read row 0 safely
    return (core_of, local_rank, dinv, idx4, msel, calls, iw_off, cols_total,
            n_local, n_sg)


# ------------------------------------------------------------- bass program
def _build_program(n_local, n_sg, calls, iw_off, cols_total, f_in, n_cls,
                   n_hidden_layers):
    """n_hidden_layers = number of 32->32 convs (6 for the real problem)."""
    F = 32
    npad = n_sg * SGN
    ntab = N_CORES * npad
    idx_words = int(iw_off[-1])
    n_layers = n_hidden_layers + 2        # conv0 + hidden + output conv
    NQ = 4                                # SWDGE queues for dma_gather

    nc = bacc.Bacc(num_devices=N_CORES, num_swdge_queues=NQ,
                   dynamic_dma_scratch_size=65536)
    xT_p = nc.declare_dram_parameter("xT", [f_in, npad], F32, False)
    idx4_p = nc.declare_dram_parameter("idx4", [P, idx_words + 8], I16, False)
    msel_p = nc.declare_dram_parameter("msel", [P, 4 * cols_total], BF16,
                                       False)
    dinvb_p = nc.declare_dram_parameter("dinvb", [P, n_sg * G * F], F32, False)
    mask_p = nc.declare_dram_parameter(
        "mask", [n_layers - 1, n_sg, P, G * F], BF16, False)
    w0_p = nc.declare_dram_parameter("W0", [f_in, F], F32, False)
    if n_hidden_layers:
        w4_p = nc.declare_dram_parameter("W4", [n_hidden_layers, P, P], F32, False)
        b4_p = nc.declare_dram_parameter("b4", [P, n_hidden_layers], F32, False)
    b0t_p = nc.declare_dram_parameter("b0t", [P, G * F], F32, False)
    wout_p = nc.declare_dram_parameter("Wout", [P, n_cls], F32, False)
    bout_p = nc.declare_dram_parameter("bout", [n_cls, 1], F32, False)
    out_p = nc.declare_dram_parameter("outT", [n_cls, npad], F32, True)

    rg = [list(range(N_CORES))]

    with tile.TileContext(nc) as tc:
        import contextlib
        with contextlib.ExitStack() as ctx:
            const = ctx.enter_context(tc.tile_pool(name="const", bufs=1))
            dram = ctx.enter_context(
                tc.tile_pool(name="dramp", bufs=1, space="DRAM"))
            psum = ctx.enter_context(
                tc.tile_pool(name="psum", bufs=4, space="PSUM"))
            pso = ctx.enter_context(
                tc.tile_pool(name="pso", bufs=2, space="PSUM"))
            pscrap = ctx.enter_context(
                tc.tile_pool(name="pscrap", bufs=1, space="PSUM"))
            sb = ctx.enter_context(tc.tile_pool(name="sb", bufs=3))
            sb2 = ctx.enter_context(tc.tile_pool(name="sb2", bufs=4))
            gat = ctx.enter_context(tc.tile_pool(name="gat", bufs=10))
            sgp = ctx.enter_context(tc.tile_pool(name="sgp", bufs=2))
            xts = ctx.enter_context(tc.tile_pool(name="xts", bufs=2))

            # persistent tiles
            idx4_sb = const.tile([P, idx_words + 8], I16, name="idx4_sb")
            nc.sync.dma_start(out=idx4_sb[:], in_=idx4_p[:])
            msel_sb = const.tile([P, 4 * cols_total], BF16, name="msel_sb")
            nc.sync.dma_start(out=msel_sb[:], in_=msel_p[:])
            dinvb = const.tile([P, n_sg * G * F], F32, name="dinvb_sb")
            nc.sync.dma_start(out=dinvb[:], in_=dinvb_p[:])
            w0_sb = const.tile([f_in, F], F32, name="w0_sb")
            nc.sync.dma_start(out=w0_sb[:], in_=w0_p[:])
            if n_hidden_layers:
                w4_sb = const.tile([P, n_hidden_layers, P], F32, name="w4_sb")
                nc.sync.dma_start(
                    out=w4_sb[:], in_=w4_p[:].rearrange("l k m -> k l m"))
                b4_sb = const.tile([P, n_hidden_layers], F32, name="b4_sb")
                nc.sync.dma_start(out=b4_sb[:], in_=b4_p[:])
            b0t_sb = const.tile([P, G * F], F32, name="b0t_sb")
            nc.sync.dma_start(out=b0t_sb[:], in_=b0t_p[:])
            wout_sb = const.tile([P, n_cls], F32, name="wout_sb")
            nc.sync.dma_start(out=wout_sb[:], in_=wout_p[:])
            bout_sb = const.tile([n_cls, 1], F32, name="bout_sb")
            nc.sync.dma_start(out=bout_sb[:], in_=bout_p[:])
            ident = const.tile([P, P], F32, name="ident_sb")
            make_identity(nc, ident[:])
            xold = const.tile([P, n_sg * G * F], F32, name="xold_sb")
            nc.vector.memset(xold[:], 0.0)
            stag = const.tile([P, n_sg * G * F], BF16, name="stag_sb")
            scrap_ps = pscrap.tile([32, 32], F32, name="scrapps_sb")
            scrap_dve = const.tile([1, 8], F32, name="scrapdve_sb")
            scrap_dve2 = const.tile([1, 8], F32, name="scrapdve2_sb")
            scrap_dve3 = const.tile([1, 8], F32, name="scrapdve3_sb")
            scrap_dve4 = const.tile([1, 8], BF16, name="scrapdve4_sb")
            scrap_act = const.tile([1, 8], F32, name="scrapact_sb")

            pool_ord = [0]

            def gq():
                q = pool_ord[0] % NQ
                pool_ord[0] += 1
                return q

            obs_idx = None     # set after idx4_sb load: zero idx window
            shard = dram.tile([npad, F], BF16, name="shard_d")
            tables = [
                dram.tile([ntab, F], BF16, name=f"tab{i}_d", addr_space="Shared")
                for i in range(n_layers)]

            # --- startup observers: absorb const-load DMA ticks per engine
            nc.tensor.transpose(out=scrap_ps[:], in_=ident[0:32, 0:32],
                                identity=ident[0:32, 0:32])
            nc.tensor.transpose(out=scrap_ps[:], in_=w0_sb[0:32, 0:32],
                                identity=ident[0:32, 0:32])
            if n_hidden_layers:
                nc.tensor.transpose(out=scrap_ps[:], in_=w4_sb[0:32, 0, 0:32],
                                    identity=ident[0:32, 0:32])
                nc.scalar.activation(out=scrap_act[:, 0:1], in_=b4_sb[0:1, 0:1],
                                     func=ACTF.Copy)
            nc.tensor.transpose(out=scrap_ps[:], in_=wout_sb[0:32, 0:32],
                                identity=ident[0:32, 0:32])
            nc.vector.tensor_copy(out=scrap_dve[:, 0:1], in_=dinvb[0:1, 0:1])
            nc.vector.tensor_copy(out=scrap_dve2[:, 0:1], in_=b0t_sb[0:1, 0:1])
            nc.vector.tensor_copy(out=scrap_dve3[:, 0:1], in_=bout_sb[0:1, 0:1])
            nc.vector.tensor_copy(out=scrap_dve4[:, 0:1], in_=msel_sb[0:1, 0:1])
            obs_idx = idx4_sb[:, idx_words : idx_words + 8]
            maskview = mask_p[:].rearrange("a b p (f2 e) -> (a b p f2) e", e=128)
            sg0 = sgp.tile([P, 128], BF16, tag="obs", name="obs_start")
            nc.gpsimd.dma_gather(
                sg0[:].rearrange("p (c e) -> p c e", e=128), maskview,
                obs_idx, P, P, 128, queue_num=gq())

            def observe_table(li, tab):
                # absorb the collective-done tick on SP and Pool
                ssp = const.tile([1, F], BF16, name=f"obs_sp{li}")
                nc.sync.dma_start(out=ssp[:], in_=tab[0:1, :])
                tab4v = tab[:].rearrange("(r q) f -> r (q f)", q=4)
                so = sgp.tile([P, 128], BF16, tag="obs", name=f"obs_pl{li}")
                nc.gpsimd.dma_gather(
                    so[:].rearrange("p (c e) -> p c e", e=128), tab4v,
                    obs_idx, P, P, 128, queue_num=gq())

            def flush_sg(sg):
                """Stage one supergroup's slice of the shard early."""
                nc.sync.dma_start(
                    out=shard[sg * SGN : (sg + 1) * SGN, :].rearrange(
                        "(g p) f -> p g f", g=G, p=P),
                    in_=stag[:, sg * G * F : (sg + 1) * G * F].rearrange(
                        "p (g f) -> p g f", f=F))

            def flush_and_gather(li):
                """AllGather the (already staged) shard into tables[li]."""
                nc.gpsimd.collective_compute(
                    "AllGather", ALU.bypass, replica_groups=rg,
                    ins=[shard.opt()], outs=[tables[li].opt()])
                observe_table(li, tables[li])

            def psg_of(sg):
                return P

            # ---------------- conv 0: h0 = x @ W0, staging <- dinv * h0
            for sg in range(n_sg):
                xt = xts.tile([f_in, SGN], F32, tag="xt", name=f"xt{sg}")
                nc.sync.dma_start(
                    out=xt[:], in_=xT_p[:, sg * SGN : (sg + 1) * SGN])
                # absorb the xt DMA tick on PE before the real matmuls
                nc.tensor.transpose(out=scrap_ps[:], in_=xt[0:32, 0:32],
                                    identity=ident[0:32, 0:32])
                ps = psum.tile([P, G * F], F32, tag="ps", name=f"c0ps{sg}")
                for g in range(G):
                    nc.tensor.matmul(
                        out=ps[:, g * F : (g + 1) * F],
                        lhsT=xt[:, g * P : (g + 1) * P],
                        rhs=w0_sb[:], start=True, stop=True)
                nc.vector.tensor_tensor(
                    out=stag[:, sg * G * F : (sg + 1) * G * F], in0=ps[:],
                    in1=dinvb[:, sg * G * F : (sg + 1) * G * F], op=ALU.mult)
                flush_sg(sg)
            flush_and_gather(0)

            # ---------------- convs 1..n_layers
            sg_call_ranges = []
            for sg in range(n_sg):
                ts = [t for t, cc in enumerate(calls) if cc[0] == sg]
                sg_call_ranges.append((min(ts), max(ts) + 1) if ts else (0, 0))
            for li in range(1, n_layers + 1):
                tab = tables[li - 1]
                tab4 = tab[:].rearrange("(r q) f -> r (q f)", q=4)
                hidden = li < n_layers
                for sg in range(n_sg):
                    psg = psg_of(sg)
                    agg = sb.tile([P, G * F], F32, tag="agg", name=f"ag{li}_{sg}")
                    t0, t1 = sg_call_ranges[sg]
                    bands = {calls[t][1] for t in range(t0, t1)}
                    for g in range(G):
                        # empty degree bands (tail supergroup): keep finite
                        if g not in bands:
                            nc.vector.memset(agg[:, g * F : (g + 1) * F], 0.0)
                    seen = set()
                    for tcall in range(t0, t1):
                        _, g, cbase, w = calls[tcall]
                        iw = int(iw_off[tcall])
                        gt = gat.tile([P, CW * 128], BF16, tag="gt",
                                      name=f"gt{li}_{tcall}")
                        nc.gpsimd.dma_gather(
                            gt[:, : w * 128].rearrange(
                                "p (c e) -> p c e", e=128),
                            tab4,
                            idx4_sb[:, iw : iw + 8 * w],
                            w * P, w * P, 128,
                            queue_num=gq())
                        prod = sb2.tile([P, CW * 128], BF16, tag="prod",
                                        name=f"pr{li}_{tcall}")
                        in0 = gt[:, : w * 128].rearrange(
                            "p (cb f) -> p cb f", f=F)
                        in1 = msel_sb[:, 4 * cbase : 4 * (cbase + w)].rearrange(
                            "p cb -> p cb ()")
                        i0b, i1b = broadcast_tensor_aps(in0, in1)
                        nc.vector.tensor_tensor(
                            out=prod[:, : w * 128].rearrange(
                                "p (cb f) -> p cb f", f=F),
                            in0=i0b, in1=i1b, op=ALU.mult)
                        # fold call halves contiguously (cb = 4w is even),
                        # then a half-size strided reduce over 2w cb-units
                        half = sb2.tile([P, CW * 64], BF16, tag="half",
                                        name=f"hf{li}_{tcall}")
                        nc.vector.tensor_tensor(
                            out=half[:, : w * 64],
                            in0=prod[:, : w * 64],
                            in1=prod[:, w * 64 : w * 128], op=ALU.add)
                        red_in = half[:, : w * 64].rearrange(
                            "p (cb f) -> p f cb", f=F)
                        if g not in seen:
                            # first call of this (sg, g) rect writes agg direct
                            seen.add(g)
                            nc.vector.tensor_reduce(
                                out=agg[:, g * F : (g + 1) * F],
                                in_=red_in, axis=AX.X, op=ALU.add)
                        else:
                            part = sb2.tile([P, F], F32, tag="part",
                                            name=f"pt{li}_{tcall}")
                            nc.vector.tensor_reduce(
                                out=part[:], in_=red_in, axis=AX.X, op=ALU.add)
                            nc.vector.tensor_tensor(
                                out=agg[:, g * F : (g + 1) * F],
                                in0=agg[:, g * F : (g + 1) * F],
                                in1=part[:], op=ALU.add)
                    dv = dinvb[:, sg * G * F : (sg + 1) * G * F]
                    r_nm = sb.tile([P, G * F], F32, tag="rnm", name=f"rn{li}_{sg}")
                    # self-loop term (staging still holds last layer's dinv*x)
                    nc.vector.tensor_tensor(
                        out=r_nm[:], in0=agg[:],
                        in1=stag[:, sg * G * F : (sg + 1) * G * F], op=ALU.add)
                    nc.vector.tensor_tensor(
                        out=r_nm[:psg], in0=r_nm[:psg], in1=dv[:psg], op=ALU.mult)
                    if li == 1:
                        # conv0 epilogue: t = relu(agg*dinv + b0); xold=t
                        nc.vector.tensor_tensor(
                            out=r_nm[:psg], in0=r_nm[:psg], in1=b0t_sb[:psg],
                            op=ALU.add)
                        xsl = xold[:psg, sg * G * F : (sg + 1) * G * F]
                        nc.vector.tensor_scalar_max(xsl, r_nm[:psg], 0.0)
                        _emit_mask_and_write(
                            nc, sb, mask_p, 0, sg, xold, dv, stag, F)
                        flush_sg(sg)
                        continue
                    # transpose r -> rT (feat-major strips)
                    nh = (G * F) // P             # transpose halves (=2)
                    pst = psum.tile([P, G * F], F32, tag="ps", name=f"pt{li}_{sg}")
                    for h in range(nh):
                        nc.tensor.transpose(
                            out=pst[:, h * P : h * P + psg],
                            in_=r_nm[:psg, h * P : (h + 1) * P],
                            identity=ident[:psg, :psg])
                    rt = sb.tile([P, G * F], F32, tag="rt", name=f"rt{li}_{sg}")
                    for h in range(nh):
                        nc.vector.tensor_copy(
                            out=rt[:, h * P : h * P + psg],
                            in_=pst[:, h * P : h * P + psg])
                    if hidden:
                        hw = li - 2
                        psh = psum.tile([P, G * F], F32, tag="ps",
                                        name=f"ph{li}_{sg}")
                        for h in range(nh):
                            nc.tensor.matmul(
                                out=psh[:, h * P : h * P + psg],
                                lhsT=w4_sb[:, hw, :],
                                rhs=rt[:, h * P : h * P + psg],
                                start=True, stop=True)
                        ht = sb.tile([P, G * F], F32, tag="ht",
                                     name=f"ht{li}_{sg}")
                        for h in range(nh):
                            nc.scalar.activation(
                                out=ht[:, h * P : h * P + psg],
                                in_=psh[:, h * P : h * P + psg],
                                func=ACTF.Relu, bias=b4_sb[:, hw : hw + 1])
                        psb = psum.tile([P, G * F], F32, tag="ps",
                                        name=f"pb{li}_{sg}")
                        for h in range(nh):
                            nc.tensor.transpose(
                                out=psb[:psg, h * P : (h + 1) * P],
                                in_=ht[:, h * P : h * P + psg],
                                identity=ident[:])
                        xsl = xold[:psg, sg * G * F : (sg + 1) * G * F]
                        nc.vector.tensor_tensor(
                            out=xsl, in0=psb[:psg], in1=xsl, op=ALU.add)
                        _emit_mask_and_write(
                            nc, sb, mask_p, li - 1, sg, xold, dv, stag, F)
                        flush_sg(sg)
                    else:
                        # output conv: out strips = Wout^T @ rT quads
                        for g in range(G):
                            h, i = g // 4, g % 4
                            if h >= nh:
                                continue
                            po = pso.tile([n_cls, P], F32, tag="po",
                                          name=f"po{sg}_{g}")
                            nc.tensor.matmul(
                                out=po[:, :psg],
                                lhsT=wout_sb[i * F : (i + 1) * F, :],
                                rhs=rt[i * F : (i + 1) * F, h * P : h * P + psg],
                                start=True, stop=True,
                                tile_position=(i * F, 0))
                            ot = sb.tile([n_cls, P], F32, tag="ot",
                                         name=f"ot{sg}_{g}")
                            nc.vector.tensor_scalar_add(
                                ot[:, :psg], po[:, :psg], bout_sb[:, 0:1])
                            nc.sync.dma_start(
                                out=out_p[:, sg * SGN + g * P : sg * SGN + g * P + psg],
                                in_=ot[:, :psg])
                if hidden:
                    flush_and_gather(li)
    if not nc.is_finalized():
        nc.finalize()
    return nc


def _emit_mask_and_write(nc, sb, mask_p, li, sg, xold, dv, stag, F):
    """xd = xold_slice * mask[li,sg]; staging <- bf16(xd * dinv).

    Full 128-partition ops: pad rows give 0 (xold memset, mask/dinv
    host-zeroed), which makes the shard pad rows exactly zero.
    """
    mt = sb.tile([P, G * F], BF16, tag="mt", name=f"mt{li}_{sg}")
    nc.sync.dma_start(out=mt[:], in_=mask_p[li, sg, :, :])
    xsl = xold[:, sg * G * F : (sg + 1) * G * F]
    xd = sb.tile([P, G * F], F32, tag="xd", name=f"xd{li}_{sg}")
    nc.vector.tensor_tensor(out=xd[:], in0=xsl, in1=mt[:], op=ALU.mult)
    nc.vector.tensor_tensor(out=stag[:, sg * G * F : (sg + 1) * G * F],
                            in0=xd[:], in1=dv, op=ALU.mult)


# ------------------------------------------------------------------ driver
def _host_inputs(x, edge_index, drop_u, W0, b0, W_hid, b_hid, W_out, b_out,
                 struct):
    (core_of, local_rank, dinv, idx4, msel, calls, iw_off, cols_total,
     n_local, n_sg) = struct
    n, f_in = x.shape
    F = 32
    n_cls = W_out.shape[1]
    nhid = W_hid.shape[0]
    npad = n_sg * SGN
    n_masks = drop_u.shape[0]

    # rank -> old id per core; xT col order (sg, g, p): col -> rank
    cols = np.arange(npad)
    csg, cj = cols // SGN, cols % SGN
    cg, cp = cj // P, cj % P
    rank_of_col = csg * SGN + cg * P + cp          # may exceed n_local (pad)
    col_valid = rank_of_col < n_local

    w4 = np.zeros((nhid, P, P), dtype=np.float32)
    for i in range(nhid):
        for q in range(4):
            w4[i, q * F : (q + 1) * F, q * F : (q + 1) * F] = W_hid[i]
    b4 = np.tile(b_hid.T, (4, 1)).astype(np.float32) if nhid else None  # [128, nhid]

    in_maps = []
    for c in range(N_CORES):
        nodes_c = np.where(core_of == c)[0]
        r = local_rank[nodes_c]
        ordmap = np.empty(n_local, dtype=np.int64)
        ordmap[r] = nodes_c

        xT = np.zeros((f_in, npad), dtype=np.float32)
        xT[:, col_valid] = x[ordmap[rank_of_col[col_valid]]].T

        dvals = np.zeros(npad, dtype=np.float32)
        dvals[:n_local] = dinv[ordmap]
        dinvb = np.repeat(
            dvals.reshape(n_sg, G, P).transpose(2, 0, 1), F,
            axis=2).reshape(P, n_sg * G * F).copy()

        mask = np.zeros((n_masks, n_sg, P, G * F), dtype=ml_dtypes.bfloat16)
        mvals = np.where(drop_u[:, ordmap, :] > 0.5, 2.0, 0.0).astype(
            ml_dtypes.bfloat16)                      # [n_masks, n_local, F]
        mpad = np.zeros((n_masks, npad, F), dtype=ml_dtypes.bfloat16)
        mpad[:, :n_local] = mvals
        mask[:] = mpad.reshape(n_masks, n_sg, G, P, F).transpose(
            0, 1, 3, 2, 4).reshape(n_masks, n_sg, P, G * F)

        im = {
            "xT": xT,
            "idx4": idx4[c],
            "msel": msel[c],
            "dinvb": dinvb,
            "mask": mask,
            "W0": W0.astype(np.float32),
            "b0t": np.tile(b0, (P, G)).astype(np.float32),
            "Wout": np.tile(W_out, (4, 1)).astype(np.float32),
            "bout": b_out.reshape(-1, 1).astype(np.float32),
        }
        if nhid:
            im["W4"] = w4
            im["b4"] = b4
        in_maps.append(im)
    return in_maps


def kernel(x, edge_index, drop_u, W0, b0, W_hid, b_hid, W_out, b_out,
           _runner=None):
    x = np.asarray(x, dtype=np.float32)
    edge_index = np.asarray(edge_index)
    drop_u = np.asarray(drop_u, dtype=np.float32)
    W0 = np.asarray(W0, dtype=np.float32)
    b0 = np.asarray(b0, dtype=np.float32)
    W_hid = np.asarray(W_hid, dtype=np.float32)
    b_hid = np.asarray(b_hid, dtype=np.float32)
    W_out = np.asarray(W_out, dtype=np.float32)
    b_out = np.asarray(b_out, dtype=np.float32)

    n, f_in = x.shape
    n_cls = W_out.shape[1]
    struct = _build_structure(edge_index, n)
    (core_of, local_rank, dinv, idx4, msel, calls, iw_off, cols_total,
     n_local, n_sg) = struct

    nc = _build_program(n_local, n_sg, calls, iw_off, cols_total, f_in,
                        n_cls, W_hid.shape[0])
    in_maps = _host_inputs(x, edge_index, drop_u, W0, b0, W_hid, b_hid,
                           W_out, b_out, struct)

    if _runner is not None:
        results = _runner(nc, in_maps)
    else:
        results = run_bass_kernel_spmd(
            nc, in_maps, core_ids=list(range(N_CORES))).results

    # un-permute: outT [n_cls, npad] per core, col -> rank -> old id
    npad = n_sg * SGN
    cols = np.arange(npad)
    csg, cj = cols // SGN, cols % SGN
    cg, cp = cj // P, cj % P
    rank_of_col = csg * SGN + cg * P + cp
    col_valid = rank_of_col < n_local

    out = np.zeros((n, n_cls), dtype=np.float32)
    for c in range(N_CORES):
        nodes_c = np.where(core_of == c)[0]
        r = local_rank[nodes_c]
        ordmap = np.empty(n_local, dtype=np.int64)
        ordmap[r] = nodes_c
        ot = np.asarray(results[c]["outT"], dtype=np.float32)  # [n_cls, npad]
        out[ordmap[rank_of_col[col_valid]]] = ot[:, col_valid].T
    return out


# revision 19
# speedup vs baseline: 1.0217x; 1.0089x over previous
"""DeepGCN (8-layer GCNConv, N=100k nodes, E=1.6M edges) on 8 Trainium2 cores.

Strategy (graph/data parallel, dst-sharded edges):
  - Nodes are degree-sorted and dealt serpentine-wise across the 8 cores so
    every core owns n/8 nodes with a near-identical degree profile.  Within a
    core, nodes are packed into "supergroups" of 1024 = 128 partitions x 8
    degree-band groups (group g = the g-th 128 nodes by degree rank), and the
    per-edge slot table is rectangular per (supergroup, group) with height =
    that band's max in-degree (uniform across cores), within 1.5% of the
    true edge count.
  - Each layer writes a bf16 feature table row per node, pre-scaled by
    dinv[node] (the src half of the GCN norm), with one all-zero pad row per
    shard.  An AllGather assembles the full table on every core.
  - Message passing: the table is viewed as 256-byte "quad rows" of 4
    consecutive nodes, and edges are fetched with dma_gather (int16 quad-row
    indices, up to 1024 per call from 128-byte-aligned index windows, spread
    over 4 SWDGE queues congruently with the Tile scheduler's 8-lane DMA
    semaphore rotation so the Q7 descriptor generation pipelines).  A per-slot {0,1} mask (broadcast along the
    feature axis) selects the right node out of each gathered quad, then a
    strided DVE tensor_reduce folds the 8 slots x 4 subblocks into the
    per-destination partial sum.  dinv[dst] is applied after the reduce.
  - GCNConv is computed aggregate-first:  A(xW) == (Ax)W.  The 32x32 weight
    is applied as a block-diagonal 4x(32x32) 128x128 matmul on PE after a
    128x128 PE transpose; bias+relu fuse into the PSUM->SBUF copy on ACT.
  - Residual + dropout-mask multiply + next-layer table write all happen in
    node-major [128, 8*32] tiles on DVE.
"""

import numpy as np
import ml_dtypes

import concourse.bass as bass
import concourse.bacc as bacc
import concourse.mybir as mybir
import concourse.tile as tile
from concourse.bass import broadcast_tensor_aps
from concourse.bass_utils import run_bass_kernel_spmd
from concourse.masks import make_identity

N_CORES = 8
P = 128
G = 8            # groups (nodes) per partition-row of one supergroup
SGN = P * G      # nodes per supergroup
CW = 8           # gather-call width: 8 slot-columns = 1024 indices
F32 = mybir.dt.float32
BF16 = mybir.dt.bfloat16
I32 = mybir.dt.int32
I16 = mybir.dt.int16
AX = mybir.AxisListType
ALU = mybir.AluOpType
ACTF = mybir.ActivationFunctionType


# ---------------------------------------------------------------- host prep
def _build_structure(edge_index, n):
    """Degree-sorted serpentine node partition + per-core slot tables.

    Slot table: per supergroup sg, per degree-band group g, kgg[sg,g] slot
    columns (col = goff[sg*G+g] + j).  Each slot holds the int16 quad-row id
    (new_id >> 2) of its source node; a per-slot one-hot bf16 mask over the
    4 subblocks encodes new_id & 3.  Slot columns are consumed by dma_gather
    in calls of <= CW=8 columns (<=1024 indices, wrapped over 16 partitions
    in 128B-aligned windows).
    """
    E = edge_index.shape[1]
    dst_all = np.concatenate([edge_index[1].astype(np.int64), np.arange(n)])
    deg = np.bincount(dst_all, minlength=n)
    dinv = (1.0 / np.sqrt(deg)).astype(np.float32)

    order = np.argsort(-deg, kind="stable")
    idx = np.arange(n)
    rounds, pos = idx // N_CORES, idx % N_CORES
    cores_seq = np.where(rounds % 2 == 0, pos, N_CORES - 1 - pos)
    core_of = np.empty(n, dtype=np.int32)
    core_of[order] = cores_seq
    local_rank = np.empty(n, dtype=np.int64)
    for c in range(N_CORES):
        nodes_c = order[cores_seq == c]
        local_rank[nodes_c] = np.arange(len(nodes_c))

    n_local = n // N_CORES
    n_sg = (n_local + SGN - 1) // SGN
    npad = n_sg * SGN
    new_id = core_of.astype(np.int64) * npad + local_rank
    pad_q = n_local >> 2                  # core 0's zero pad region, quad row

    edge_dst = edge_index[1].astype(np.int64)
    edge_src = edge_index[0].astype(np.int64)
    per_core = []
    kmat = np.zeros((N_CORES, n_sg), dtype=np.int64)
    for c in range(N_CORES):
        em = core_of[edge_dst] == c
        e_src = new_id[edge_src[em]]
        e_rank = local_rank[edge_dst[em]]
        o = np.argsort(e_rank, kind="stable")
        e_src, e_rank = e_src[o], e_rank[o]
        counts = np.bincount(e_rank, minlength=n_local)
        starts = np.concatenate([[0], np.cumsum(counts)])
        per_core.append((e_src, e_rank, counts, starts))
        for sg in range(n_sg):
            kmat[c, sg] = counts[sg * SGN : min((sg + 1) * SGN, n_local)].max()
    # per-(sg, g) rectangle heights, max over the 128 dsts and all cores
    kgg = np.zeros((N_CORES, n_sg, G), dtype=np.int64)
    for c in range(N_CORES):
        _, e_rank, _, _ = per_core[c]
        sg = e_rank // SGN
        i = e_rank % SGN
        key = (sg * G + (i // P)) * P + (i % P)
        cnt = np.bincount(key, minlength=n_sg * G * P).reshape(n_sg * G, P)
        kgg[c] = cnt.max(axis=1).reshape(n_sg, G)
    kgg = kgg.max(axis=0)                        # [n_sg, G], SPMD-uniform
    goff = np.concatenate([[0], np.cumsum(kgg.ravel())]).reshape(-1)
    cols_total = int(goff[-1])

    # call list: per (sg, g), chunks of <= CW columns; width w -> 128*w idxs
    calls = []                                   # (sg, g, colbase, width)
    for sg in range(n_sg):
        for g in range(G):
            k = int(kgg[sg, g])
            base = int(goff[sg * G + g])
            off = 0
            while off < k:
                w = min(CW, k - off)
                calls.append((sg, g, base + off, w))
                off += w
    iw_off = np.arange(len(calls) + 1) * 64      # 128B-aligned idx windows
    idx_words = int(iw_off[-1])                  # int16 words per partition

    slots4 = np.full((N_CORES, P, cols_total), pad_q, dtype=np.int16)
    msel = np.zeros((N_CORES, P, 4 * cols_total), dtype=ml_dtypes.bfloat16)
    idx4 = np.zeros((N_CORES, P, idx_words + 8), dtype=np.int16)
    for c in range(N_CORES):
        e_src, e_rank, counts, starts = per_core[c]
        j = np.arange(len(e_src)) - starts[e_rank]      # slot within node
        sg = e_rank // SGN
        i = e_rank % SGN
        pp, gg = i % P, i // P
        col = goff[sg * G + gg] + j
        slots4[c, pp, col] = (e_src >> 2).astype(np.int16)
        msel[c, pp, 4 * col + (e_src & 3)] = 1.0
        # wrapped indices per call: position j = colrel*128 + p -> [j%16, j//16]
        blk = np.zeros((16, idx_words), dtype=np.int16)
        for t, (csg, cg, cbase, w) in enumerate(calls):
            arr = slots4[c, :, cbase : cbase + w].T.ravel()   # [128*w]
            blk[:, int(iw_off[t]) : int(iw_off[t]) + 8 * w] = (
                arr.reshape(8 * w, 16).T)
        idx4[c, :, :idx_words] = np.tile(blk, (8, 1))
        # trailing all-zero window: observer gathers read row 0 safely
    return (core_of, local_rank, dinv, idx4, msel, calls, iw_off, cols_total,
            n_local, n_sg)


# ------------------------------------------------------------- bass program
def _build_program(n_local, n_sg, calls, iw_off, cols_total, f_in, n_cls,
                   n_hidden_layers):
    """n_hidden_layers = number of 32->32 convs (6 for the real problem)."""
    F = 32
    npad = n_sg * SGN
    ntab = N_CORES * npad
    idx_words = int(iw_off[-1])
    n_layers = n_hidden_layers + 2        # conv0 + hidden + output conv
    NQ = 4                                # SWDGE queues for dma_gather

    nc = bacc.Bacc(num_devices=N_CORES, num_swdge_queues=NQ,
                   dynamic_dma_scratch_size=65536)
    xT_p = nc.declare_dram_parameter("xT", [f_in, npad], F32, False)
    idx4_p = nc.declare_dram_parameter("idx4", [P, idx_words + 8], I16, False)
    msel_p = nc.declare_dram_parameter("msel", [P, 4 * cols_total], BF16,
                                       False)
    dinvb_p = nc.declare_dram_parameter("dinvb", [P, n_sg * G * F], F32, False)
    mask_p = nc.declare_dram_parameter(
        "mask", [n_layers - 1, n_sg, P, G * F], BF16, False)
    w0_p = nc.declare_dram_parameter("W0", [f_in, F], F32, False)
    if n_hidden_layers:
        w4_p = nc.declare_dram_parameter("W4", [n_hidden_layers, P, P], F32, False)
        b4_p = nc.declare_dram_parameter("b4", [P, n_hidden_layers], F32, False)
    b0t_p = nc.declare_dram_parameter("b0t", [P, G * F], F32, False)
    wout_p = nc.declare_dram_parameter("Wout", [P, n_cls], F32, False)
    bout_p = nc.declare_dram_parameter("bout", [n_cls, 1], F32, False)
    out_p = nc.declare_dram_parameter("outT", [n_cls, npad], F32, True)

    rg = [list(range(N_CORES))]

    with tile.TileContext(nc) as tc:
        import contextlib
        with contextlib.ExitStack() as ctx:
            const = ctx.enter_context(tc.tile_pool(name="const", bufs=1))
            dram = ctx.enter_context(
                tc.tile_pool(name="dramp", bufs=1, space="DRAM"))
            psum = ctx.enter_context(
                tc.tile_pool(name="psum", bufs=4, space="PSUM"))
            pso = ctx.enter_context(
                tc.tile_pool(name="pso", bufs=2, space="PSUM"))
            pscrap = ctx.enter_context(
                tc.tile_pool(name="pscrap", bufs=1, space="PSUM"))
            sb = ctx.enter_context(tc.tile_pool(name="sb", bufs=3))
            sb2 = ctx.enter_context(tc.tile_pool(name="sb2", bufs=4))
            gat = ctx.enter_context(tc.tile_pool(name="gat", bufs=10))
            sgp = ctx.enter_context(tc.tile_pool(name="sgp", bufs=2))
            xts = ctx.enter_context(tc.tile_pool(name="xts", bufs=2))

            # persistent tiles
            idx4_sb = const.tile([P, idx_words + 8], I16, name="idx4_sb")
            nc.sync.dma_start(out=idx4_sb[:], in_=idx4_p[:])
            msel_sb = const.tile([P, 4 * cols_total], BF16, name="msel_sb")
            nc.sync.dma_start(out=msel_sb[:], in_=msel_p[:])
            dinvb = const.tile([P, n_sg * G * F], F32, name="dinvb_sb")
            nc.sync.dma_start(out=dinvb[:], in_=dinvb_p[:])
            w0_sb = const.tile([f_in, F], F32, name="w0_sb")
            nc.sync.dma_start(out=w0_sb[:], in_=w0_p[:])
            if n_hidden_layers:
                w4_sb = const.tile([P, n_hidden_layers, P], F32, name="w4_sb")
                nc.sync.dma_start(
                    out=w4_sb[:], in_=w4_p[:].rearrange("l k m -> k l m"))
                b4_sb = const.tile([P, n_hidden_layers], F32, name="b4_sb")
                nc.sync.dma_start(out=b4_sb[:], in_=b4_p[:])
            b0t_sb = const.tile([P, G * F], F32, name="b0t_sb")
            nc.sync.dma_start(out=b0t_sb[:], in_=b0t_p[:])
            wout_sb = const.tile([P, n_cls], F32, name="wout_sb")
            nc.sync.dma_start(out=wout_sb[:], in_=wout_p[:])
            bout_sb = const.tile([n_cls, 1], F32, name="bout_sb")
            nc.sync.dma_start(out=bout_sb[:], in_=bout_p[:])
            ident = const.tile([P, P], F32, name="ident_sb")
            make_identity(nc, ident[:])
            xold = const.tile([P, n_sg * G * F], F32, name="xold_sb")
            nc.vector.memset(xold[:], 0.0)
            stag = const.tile([P, n_sg * G * F], BF16, name="stag_sb")
            scrap_ps = pscrap.tile([32, 32], F32, name="scrapps_sb")
            scrap_dve = const.tile([1, 8], F32, name="scrapdve_sb")
            scrap_dve2 = const.tile([1, 8], F32, name="scrapdve2_sb")
            scrap_dve3 = const.tile([1, 8], F32, name="scrapdve3_sb")
            scrap_dve4 = const.tile([1, 8], BF16, name="scrapdve4_sb")
            scrap_act = const.tile([1, 8], F32, name="scrapact_sb")

            pool_ord = [0]

            def gq():
                q = pool_ord[0] % NQ
                pool_ord[0] += 1
                return q

            obs_idx = None     # set after idx4_sb load: zero idx window
            shard = dram.tile([npad, F], BF16, name="shard_d")
            tables = [
                dram.tile([ntab, F], BF16, name=f"tab{i}_d", addr_space="Shared")
                for i in range(n_layers)]

            # --- startup observers: absorb const-load DMA ticks per engine
            nc.tensor.transpose(out=scrap_ps[:], in_=ident[0:32, 0:32],
                                identity=ident[0:32, 0:32])
            nc.tensor.transpose(out=scrap_ps[:], in_=w0_sb[0:32, 0:32],
                                identity=ident[0:32, 0:32])
            if n_hidden_layers:
                nc.tensor.transpose(out=scrap_ps[:], in_=w4_sb[0:32, 0, 0:32],
                                    identity=ident[0:32, 0:32])
                nc.scalar.activation(out=scrap_act[:, 0:1], in_=b4_sb[0:1, 0:1],
                                     func=ACTF.Copy)
            nc.tensor.transpose(out=scrap_ps[:], in_=wout_sb[0:32, 0:32],
                                identity=ident[0:32, 0:32])
            nc.vector.tensor_copy(out=scrap_dve[:, 0:1], in_=dinvb[0:1, 0:1])
            nc.vector.tensor_copy(out=scrap_dve2[:, 0:1], in_=b0t_sb[0:1, 0:1])
            nc.vector.tensor_copy(out=scrap_dve3[:, 0:1], in_=bout_sb[0:1, 0:1])
            nc.vector.tensor_copy(out=scrap_dve4[:, 0:1], in_=msel_sb[0:1, 0:1])
            obs_idx = idx4_sb[:, idx_words : idx_words + 8]
            maskview = mask_p[:].rearrange("a b p (f2 e) -> (a b p f2) e", e=128)
            sg0 = sgp.tile([P, 128], BF16, tag="obs", name="obs_start")
            nc.gpsimd.dma_gather(
                sg0[:].rearrange("p (c e) -> p c e", e=128), maskview,
                obs_idx, P, P, 128, queue_num=gq())

            def observe_table(li, tab):
                # absorb the collective-done tick on SP and Pool
                ssp = const.tile([1, F], BF16, name=f"obs_sp{li}")
                nc.sync.dma_start(out=ssp[:], in_=tab[0:1, :])
                tab4v = tab[:].rearrange("(r q) f -> r (q f)", q=4)
                so = sgp.tile([P, 128], BF16, tag="obs", name=f"obs_pl{li}")
                nc.gpsimd.dma_gather(
                    so[:].rearrange("p (c e) -> p c e", e=128), tab4v,
                    obs_idx, P, P, 128, queue_num=gq())

            def flush_sg(sg):
                """Stage one supergroup's slice of the shard early."""
                nc.sync.dma_start(
                    out=shard[sg * SGN : (sg + 1) * SGN, :].rearrange(
                        "(g p) f -> p g f", g=G, p=P),
                    in_=stag[:, sg * G * F : (sg + 1) * G * F].rearrange(
                        "p (g f) -> p g f", f=F))

            def flush_and_gather(li):
                """AllGather the (already staged) shard into tables[li]."""
                nc.gpsimd.collective_compute(
                    "AllGather", ALU.bypass, replica_groups=rg,
                    ins=[shard.opt()], outs=[tables[li].opt()])
                observe_table(li, tables[li])

            def psg_of(sg):
                return P

            # ---------------- conv 0: h0 = x @ W0, staging <- dinv * h0
            for sg in range(n_sg):
                xt = xts.tile([f_in, SGN], F32, tag="xt", name=f"xt{sg}")
                nc.sync.dma_start(
                    out=xt[:], in_=xT_p[:, sg * SGN : (sg + 1) * SGN])
                # absorb the xt DMA tick on PE before the real matmuls
                nc.tensor.transpose(out=scrap_ps[:], in_=xt[0:32, 0:32],
                                    identity=ident[0:32, 0:32])
                ps = psum.tile([P, G * F], F32, tag="ps", name=f"c0ps{sg}")
                for g in range(G):
                    nc.tensor.matmul(
                        out=ps[:, g * F : (g + 1) * F],
                        lhsT=xt[:, g * P : (g + 1) * P],
                        rhs=w0_sb[:], start=True, stop=True)
                nc.vector.tensor_tensor(
                    out=stag[:, sg * G * F : (sg + 1) * G * F], in0=ps[:],
                    in1=dinvb[:, sg * G * F : (sg + 1) * G * F], op=ALU.mult)
                flush_sg(sg)
            flush_and_gather(0)

            # ---------------- convs 1..n_layers
            sg_call_ranges = []
            for sg in range(n_sg):
                ts = [t for t, cc in enumerate(calls) if cc[0] == sg]
                sg_call_ranges.append((min(ts), max(ts) + 1) if ts else (0, 0))
            for li in range(1, n_layers + 1):
                tab = tables[li - 1]
                tab4 = tab[:].rearrange("(r q) f -> r (q f)", q=4)
                hidden = li < n_layers
                for sg in range(n_sg):
                    psg = psg_of(sg)
                    agg = sb.tile([P, G * F], F32, tag="agg", name=f"ag{li}_{sg}")
                    t0, t1 = sg_call_ranges[sg]
                    bands = {calls[t][1] for t in range(t0, t1)}
                    for g in range(G):
                        # empty degree bands (tail supergroup): keep finite
                        if g not in bands:
                            nc.vector.memset(agg[:, g * F : (g + 1) * F], 0.0)
                    seen = set()
                    for tcall in range(t0, t1):
                        _, g, cbase, w = calls[tcall]
                        iw = int(iw_off[tcall])
                        gt = gat.tile([P, CW * 128], BF16, tag="gt",
                                      name=f"gt{li}_{tcall}")
                        nc.gpsimd.dma_gather(
                            gt[:, : w * 128].rearrange(
                                "p (c e) -> p c e", e=128),
                            tab4,
                            idx4_sb[:, iw : iw + 8 * w],
                            w * P, w * P, 128,
                            queue_num=gq())
                        prod = sb2.tile([P, CW * 128], BF16, tag="prod",
                                        name=f"pr{li}_{tcall}")
                        in0 = gt[:, : w * 128].rearrange(
                            "p (cb f) -> p cb f", f=F)
                        in1 = msel_sb[:, 4 * cbase : 4 * (cbase + w)].rearrange(
                            "p cb -> p cb ()")
                        i0b, i1b = broadcast_tensor_aps(in0, in1)
                        nc.vector.tensor_tensor(
                            out=prod[:, : w * 128].rearrange(
                                "p (cb f) -> p cb f", f=F),
                            in0=i0b, in1=i1b, op=ALU.mult)
                        # fold call halves contiguously (cb = 4w is even),
                        # then a half-size strided reduce over 2w cb-units
                        half = sb2.tile([P, CW * 64], BF16, tag="half",
                                        name=f"hf{li}_{tcall}")
                        nc.vector.tensor_tensor(
                            out=half[:, : w * 64],
                            in0=prod[:, : w * 64],
                            in1=prod[:, w * 64 : w * 128], op=ALU.add)
                        qrt = sb2.tile([P, CW * 32], BF16, tag="qrt",
                                       name=f"qr{li}_{tcall}")
                        nc.vector.tensor_tensor(
                            out=qrt[:, : w * 32],
                            in0=half[:, : w * 32],
                            in1=half[:, w * 32 : w * 64], op=ALU.add)
                        red_in = qrt[:, : w * 32].rearrange(
                            "p (cb f) -> p f cb", f=F)
                        if g not in seen:
                            # first call of this (sg, g) rect writes agg direct
                            seen.add(g)
                            nc.vector.tensor_reduce(
                                out=agg[:, g * F : (g + 1) * F],
                                in_=red_in, axis=AX.X, op=ALU.add)
                        else:
                            part = sb2.tile([P, F], F32, tag="part",
                                            name=f"pt{li}_{tcall}")
                            nc.vector.tensor_reduce(
                                out=part[:], in_=red_in, axis=AX.X, op=ALU.add)
                            nc.vector.tensor_tensor(
                                out=agg[:, g * F : (g + 1) * F],
                                in0=agg[:, g * F : (g + 1) * F],
                                in1=part[:], op=ALU.add)
                    dv = dinvb[:, sg * G * F : (sg + 1) * G * F]
                    r_nm = sb.tile([P, G * F], F32, tag="rnm", name=f"rn{li}_{sg}")
                    # self-loop term (staging still holds last layer's dinv*x)
                    nc.vector.tensor_tensor(
                        out=r_nm[:], in0=agg[:],
                        in1=stag[:, sg * G * F : (sg + 1) * G * F], op=ALU.add)
                    nc.vector.tensor_tensor(
                        out=r_nm[:psg], in0=r_nm[:psg], in1=dv[:psg], op=ALU.mult)
                    if li == 1:
                        # conv0 epilogue: t = relu(agg*dinv + b0); xold=t
                        nc.vector.tensor_tensor(
                            out=r_nm[:psg], in0=r_nm[:psg], in1=b0t_sb[:psg],
                            op=ALU.add)
                        xsl = xold[:psg, sg * G * F : (sg + 1) * G * F]
                        nc.vector.tensor_scalar_max(xsl, r_nm[:psg], 0.0)
                        _emit_mask_and_write(
                            nc, sb, mask_p, 0, sg, xold, dv, stag, F)
                        flush_sg(sg)
                        continue
                    # transpose r -> rT (feat-major strips)
                    nh = (G * F) // P             # transpose halves (=2)
                    pst = psum.tile([P, G * F], F32, tag="ps", name=f"pt{li}_{sg}")
                    for h in range(nh):
                        nc.tensor.transpose(
                            out=pst[:, h * P : h * P + psg],
                            in_=r_nm[:psg, h * P : (h + 1) * P],
                            identity=ident[:psg, :psg])
                    rt = sb.tile([P, G * F], F32, tag="rt", name=f"rt{li}_{sg}")
                    for h in range(nh):
                        nc.vector.tensor_copy(
                            out=rt[:, h * P : h * P + psg],
                            in_=pst[:, h * P : h * P + psg])
                    if hidden:
                        hw = li - 2
                        psh = psum.tile([P, G * F], F32, tag="ps",
                                        name=f"ph{li}_{sg}")
                        for h in range(nh):
                            nc.tensor.matmul(
                                out=psh[:, h * P : h * P + psg],
                                lhsT=w4_sb[:, hw, :],
                                rhs=rt[:, h * P : h * P + psg],
                                start=True, stop=True)
                        ht = sb.tile([P, G * F], F32, tag="ht",
                                     name=f"ht{li}_{sg}")
                        for h in range(nh):
                            nc.scalar.activation(
                                out=ht[:, h * P : h * P + psg],
                                in_=psh[:, h * P : h * P + psg],
                                func=ACTF.Relu, bias=b4_sb[:, hw : hw + 1])
                        psb = psum.tile([P, G * F], F32, tag="ps",
                                        name=f"pb{li}_{sg}")
                        for h in range(nh):
                            nc.tensor.transpose(
                                out=psb[:psg, h * P : (h + 1) * P],
                                in_=ht[:, h * P : h * P + psg],
                                identity=ident[:])
                        xsl = xold[:psg, sg * G * F : (sg + 1) * G * F]
                        nc.vector.tensor_tensor(
                            out=xsl, in0=psb[:psg], in1=xsl, op=ALU.add)
                        _emit_mask_and_write(
                            nc, sb, mask_p, li - 1, sg, xold, dv, stag, F)
                        flush_sg(sg)
                    else:
                        # output conv: out strips = Wout^T @ rT quads
                        for g in range(G):
                            h, i = g // 4, g % 4
                            if h >= nh:
                                continue
                            po = pso.tile([n_cls, P], F32, tag="po",
                                          name=f"po{sg}_{g}")
                            nc.tensor.matmul(
                                out=po[:, :psg],
                                lhsT=wout_sb[i * F : (i + 1) * F, :],
                                rhs=rt[i * F : (i + 1) * F, h * P : h * P + psg],
                                start=True, stop=True,
                                tile_position=(i * F, 0))
                            ot = sb.tile([n_cls, P], F32, tag="ot",
                                         name=f"ot{sg}_{g}")
                            nc.vector.tensor_scalar_add(
                                ot[:, :psg], po[:, :psg], bout_sb[:, 0:1])
                            nc.sync.dma_start(
                                out=out_p[:, sg * SGN + g * P : sg * SGN + g * P + psg],
                                in_=ot[:, :psg])
                if hidden:
                    flush_and_gather(li)
    if not nc.is_finalized():
        nc.finalize()
    return nc


def _emit_mask_and_write(nc, sb, mask_p, li, sg, xold, dv, stag, F):
    """xd = xold_slice * mask[li,sg]; staging <- bf16(xd * dinv).

    Full 128-partition ops: pad rows give 0 (xold memset, mask/dinv
    host-zeroed), which makes the shard pad rows exactly zero.
    """
    mt = sb.tile([P, G * F], BF16, tag="mt", name=f"mt{li}_{sg}")
    nc.sync.dma_start(out=mt[:], in_=mask_p[li, sg, :, :])
    xsl = xold[:, sg * G * F : (sg + 1) * G * F]
    xd = sb.tile([P, G * F], F32, tag="xd", name=f"xd{li}_{sg}")
    nc.vector.tensor_tensor(out=xd[:], in0=xsl, in1=mt[:], op=ALU.mult)
    nc.vector.tensor_tensor(out=stag[:, sg * G * F : (sg + 1) * G * F],
                            in0=xd[:], in1=dv, op=ALU.mult)


# ------------------------------------------------------------------ driver
def _host_inputs(x, edge_index, drop_u, W0, b0, W_hid, b_hid, W_out, b_out,
                 struct):
    (core_of, local_rank, dinv, idx4, msel, calls, iw_off, cols_total,
     n_local, n_sg) = struct
    n, f_in = x.shape
    F = 32
    n_cls = W_out.shape[1]
    nhid = W_hid.shape[0]
    npad = n_sg * SGN
    n_masks = drop_u.shape[0]

    # rank -> old id per core; xT col order (sg, g, p): col -> rank
    cols = np.arange(npad)
    csg, cj = cols // SGN, cols % SGN
    cg, cp = cj // P, cj % P
    rank_of_col = csg * SGN + cg * P + cp          # may exceed n_local (pad)
    col_valid = rank_of_col < n_local

    w4 = np.zeros((nhid, P, P), dtype=np.float32)
    for i in range(nhid):
        for q in range(4):
            w4[i, q * F : (q + 1) * F, q * F : (q + 1) * F] = W_hid[i]
    b4 = np.tile(b_hid.T, (4, 1)).astype(np.float32) if nhid else None  # [128, nhid]

    in_maps = []
    for c in range(N_CORES):
        nodes_c = np.where(core_of == c)[0]
        r = local_rank[nodes_c]
        ordmap = np.empty(n_local, dtype=np.int64)
        ordmap[r] = nodes_c

        xT = np.zeros((f_in, npad), dtype=np.float32)
        xT[:, col_valid] = x[ordmap[rank_of_col[col_valid]]].T

        dvals = np.zeros(npad, dtype=np.float32)
        dvals[:n_local] = dinv[ordmap]
        dinvb = np.repeat(
            dvals.reshape(n_sg, G, P).transpose(2, 0, 1), F,
            axis=2).reshape(P, n_sg * G * F).copy()

        mask = np.zeros((n_masks, n_sg, P, G * F), dtype=ml_dtypes.bfloat16)
        mvals = np.where(drop_u[:, ordmap, :] > 0.5, 2.0, 0.0).astype(
            ml_dtypes.bfloat16)                      # [n_masks, n_local, F]
        mpad = np.zeros((n_masks, npad, F), dtype=ml_dtypes.bfloat16)
        mpad[:, :n_local] = mvals
        mask[:] = mpad.reshape(n_masks, n_sg, G, P, F).transpose(
            0, 1, 3, 2, 4).reshape(n_masks, n_sg, P, G * F)

        im = {
            "xT": xT,
            "idx4": idx4[c],
            "msel": msel[c],
            "dinvb": dinvb,
            "mask": mask,
            "W0": W0.astype(np.float32),
            "b0t": np.tile(b0, (P, G)).astype(np.float32),
            "Wout": np.tile(W_out, (4, 1)).astype(np.float32),
            "bout": b_out.reshape(-1, 1).astype(np.float32),
        }
        if nhid:
            im["W4"] = w4
            im["b4"] = b4
        in_maps.append(im)
    return in_maps


def kernel(x, edge_index, drop_u, W0, b0, W_hid, b_hid, W_out, b_out,
           _runner=None):
    x = np.asarray(x, dtype=np.float32)
    edge_index = np.asarray(edge_index)
    drop_u = np.asarray(drop_u, dtype=np.float32)
    W0 = np.asarray(W0, dtype=np.float32)
    b0 = np.asarray(b0, dtype=np.float32)
    W_hid = np.asarray(W_hid, dtype=np.float32)
    b_hid = np.asarray(b_hid, dtype=np.float32)
    W_out = np.asarray(W_out, dtype=np.float32)
    b_out = np.asarray(b_out, dtype=np.float32)

    n, f_in = x.shape
    n_cls = W_out.shape[1]
    struct = _build_structure(edge_index, n)
    (core_of, local_rank, dinv, idx4, msel, calls, iw_off, cols_total,
     n_local, n_sg) = struct

    nc = _build_program(n_local, n_sg, calls, iw_off, cols_total, f_in,
                        n_cls, W_hid.shape[0])
    in_maps = _host_inputs(x, edge_index, drop_u, W0, b0, W_hid, b_hid,
                           W_out, b_out, struct)

    if _runner is not None:
        results = _runner(nc, in_maps)
    else:
        results = run_bass_kernel_spmd(
            nc, in_maps, core_ids=list(range(N_CORES))).results

    # un-permute: outT [n_cls, npad] per core, col -> rank -> old id
    npad = n_sg * SGN
    cols = np.arange(npad)
    csg, cj = cols // SGN, cols % SGN
    cg, cp = cj // P, cj % P
    rank_of_col = csg * SGN + cg * P + cp
    col_valid = rank_of_col < n_local

    out = np.zeros((n, n_cls), dtype=np.float32)
    for c in range(N_CORES):
        nodes_c = np.where(core_of == c)[0]
        r = local_rank[nodes_c]
        ordmap = np.empty(n_local, dtype=np.int64)
        ordmap[r] = nodes_c
        ot = np.asarray(results[c]["outT"], dtype=np.float32)  # [n_cls, npad]
        out[ordmap[rank_of_col[col_valid]]] = ot[:, col_valid].T
    return out
